# revision 79
# baseline (speedup 1.0000x reference)
"""GQA causal-attention prefill kernel for 8 Trainium2 NeuronCores.

Reference computation (B=2, S=2048, D=4096, Q=32 q-heads, N=8 kv-heads,
H=128): QKV projection + RoPE + causal GQA attention + O projection.

Sharding: core c handles batch b = c//4 and kv-head pair g = c%4
(kv-heads 2g..2g+1, q-heads 8g..8g+7).  No collectives: each core
computes its partial o-projection (sum over its 8 q-heads) and the host
sums the four partials per batch at gather time (the "all-reduce").

Device-side layout strategy (per core):
  - x is fed pre-transposed ([D, S]) so projections contract over D with
    matmuls (stationary = weight / xT tile, moving N = s-chunk).
  - q, k are produced in [h, s] layout; v in [t, h] layout.
  - scores are computed transposed (S^T = K^T Q, psum [t, s]) so the
    softmax weights feed the AV matmul directly as the moving operand
    with t as the contraction partition - no transposes anywhere.
  - softmax denominators: exp tiles accumulate on DVE in bf16, then one
    ones-matmul per (head, s-tile) does the partition-dim sum; exp on
    ScalarE (fused 1/sqrt(H) scaling); causal masking via a triangular
    additive tile, with left-of-diagonal columns never computed at all.
  - attention inner loop is software-pipelined depth-3 (scores run ahead
    of AV) with diagonal tiles first; head tails (AV drain + normalize)
    are deferred behind the next block's q-projection so the PE never
    waits on exp latency.
  - RoPE in [h, s] layout: sign-folded sin table, rotate-half done as a
    TensorE matmul with a constant permutation matrix, then one add.
  - normalization happens in pass B: 1/den via fast-approx reciprocal,
    broadcast to 128 partitions with a rank-1 ones matmul, multiplied
    into a persistent SBUF o tile that pass C consumes directly.
  - all matmul operands are bf16 (PSUM accumulation stays fp32).
"""

import math
import sys

import numpy as np

for _p in ("/opt/trn_rl_repo", "/root/.axon_site/_ro/trn_rl_repo"):
    if _p not in sys.path:
        sys.path.append(_p)

import concourse.bacc as bacc
import concourse.mybir as mybir
import concourse.tile as tile
from concourse import bass_utils

dt = mybir.dt
F32 = dt.float32
F32R = dt.float32r
BF16 = dt.bfloat16
ADD = mybir.AluOpType.add
MULT = mybir.AluOpType.mult
EXP = mybir.ActivationFunctionType.Exp
COPY = mybir.ActivationFunctionType.Copy

# Full-problem config (per core after sharding).
FULL_CFG = dict(S=2048, D=4096, QH=8, KH=2, H=128, SC=512, ST=512, HG=4)
N_CORES = 8
ROPE_THETA = 10000.0
NEG_BIG = -1.0e30


def build_bass(cfg):
    S, D, QH, KH, H = cfg["S"], cfg["D"], cfg["QH"], cfg["KH"], cfg["H"]
    SC, ST, HG = cfg["SC"], cfg["ST"], cfg["HG"]
    assert H == 128 and D % 128 == 0 and S % SC == 0 and SC % 128 == 0
    assert S % ST == 0 and ST % 128 == 0 and QH % KH == 0 and QH % HG == 0
    DT = D // 128          # d-tiles (contraction tiles for projections)
    NCH = S // SC          # s-chunks for projections
    NJ = S // ST           # s-tiles for attention
    TJ = ST // 128         # 128-wide t-tiles per attention s-tile
    NT = S // 128          # total t-tiles
    G = QH // KH           # GQA group size
    EW = 512               # o-proj output tile width
    NE = D // EW
    scale = 1.0 / math.sqrt(H)

    from contextlib import ExitStack

    nc = bacc.Bacc("TRN2", target_bir_lowering=False, debug=False,
                   enable_asserts=False, num_devices=N_CORES)

    xT = nc.dram_tensor("xT", [128, S // 512, D // 128, 512], BF16,
                        kind="ExternalInput")
    wq = nc.dram_tensor("wq", [QH, 128, D // 128, H], BF16,
                        kind="ExternalInput")
    wk = nc.dram_tensor("wk", [KH, 128, D // 128, H], BF16,
                        kind="ExternalInput")
    wv = nc.dram_tensor("wv", [KH, 128, D // 128, H], BF16,
                        kind="ExternalInput")
    wo = nc.dram_tensor("wo", [QH, H, D], BF16, kind="ExternalInput")
    cos_d = nc.dram_tensor("cos_t", [128, S], BF16, kind="ExternalInput")
    sin_d = nc.dram_tensor("sin_t", [128, S], BF16, kind="ExternalInput")
    tri_d = nc.dram_tensor("tri_t", [128, 128], F32, kind="ExternalInput")
    ones_d = nc.dram_tensor("ones_t", [128, 128], BF16, kind="ExternalInput")
    perm_d = nc.dram_tensor("perm_t", [128, 128], BF16, kind="ExternalInput")
    o_out = nc.dram_tensor("o_out", [S, D], BF16, kind="ExternalOutput")


    with tile.TileContext(nc) as tc, \
         nc.allow_low_precision(reason="deliberate fp32r matmul pipeline"):
        with tc.tile_pool(name="persist", bufs=1) as persist, \
             tc.tile_pool(name="wop", bufs=2) as wop, \
             tc.tile_pool(name="drsc", bufs=1, space="DRAM") as dram:
            cos_sb = persist.tile([128, S], BF16)
            sin_sb = persist.tile([128, S], BF16)
            tri_sb = persist.tile([128, 128], F32)
            ones_sb = persist.tile([128, 128], BF16)
            perm_sb = persist.tile([128, 128], BF16)
            k_sb = persist.tile([128, KH, S], BF16)
            v_sb = persist.tile([128, NT, KH * H], BF16)
            o_sb = persist.tile([128, QH, S], BF16)
            ones_col = ones_sb[:, 0:1]
            ones_row = ones_sb[0:1, :]

            def load_tables(lo, hi, consts=False):
                # cos/sin split by column range so only the first chunk's
                # slice rides the startup DMA burst.
                nc.sync.dma_start(cos_sb[:, lo:hi], cos_d[:, lo:hi])
                nc.sync.dma_start(sin_sb[:, lo:hi], sin_d[:, lo:hi])
                if consts:
                    nc.sync.dma_start(tri_sb[:], tri_d[:, :])
                    nc.sync.dma_start(ones_sb[:], ones_d[:, :])
                    nc.sync.dma_start(perm_sb[:], perm_d[:, :])

            def rope(ps_tile, dst_ap, s0, W, rp, swp, swtag):
                """dst = rope(ps_tile) for s-range [s0, s0+W).

                The rotate-half partition swap runs on TensorE as a
                matmul with a constant permutation matrix (sign folded
                into the sin table), keeping the chain DMA-free."""
                ta = rp.tile([128, W], F32, tag="ta")
                tb = rp.tile([128, W], BF16, tag="tb")
                csl = cos_sb[:, s0:s0 + W]
                ssl = sin_sb[:, s0:s0 + W]
                nc.vector.tensor_tensor(ta[:], ps_tile, csl, MULT)
                nc.vector.tensor_tensor(tb[:], ps_tile, ssl, MULT)
                tbs = swp.tile([128, W], F32, tag=swtag)
                nc.tensor.matmul(tbs[:], perm_sb[:], tb[:],
                                 start=True, stop=True)
                nc.vector.tensor_tensor(dst_ap, ta[:], tbs[:], ADD)

            # wq head tiles live in their own pool opened before pass A
            # so the first q-weight load overlaps k/v projection.
            wqp_es = ExitStack()
            wqp0 = wqp_es.enter_context(tc.tile_pool(name="wqp0", bufs=1))
            wq_pre = wqp0.tile([128, DT, H], BF16, tag="wq0", name="wq_pre")
            xtp_es = ExitStack()
            xtsp = xtp_es.enter_context(tc.tile_pool(name="xts", bufs=2))

            # ---- PASS A: k and v projections (+ RoPE on k) ----
            with nc.named_scope("passA"), \
                 tc.tile_pool(name="wkv", bufs=1) as wkvp, \
                 tc.tile_pool(name="ropeA", bufs=2) as rpA, \
                 tc.tile_pool(name="pskA", bufs=3, space="PSUM") as psk, \
                 tc.tile_pool(name="psvA", bufs=3, space="PSUM") as psv, \
                 tc.tile_pool(name="pswA", bufs=2, space="PSUM") as psw:
                wk_t = wkvp.tile([128, KH, DT, H], BF16)
                wv_t = wkvp.tile([128, DT, KH, H], BF16)
                first_loads_done = False
                for ch in range(NCH):
                    xts = xtsp.tile([128, DT, SC], BF16, tag="xts")
                    if not first_loads_done:
                        first_loads_done = True
                        cuts = [0, 1, 4, 12, DT]
                        for piece in range(4):
                            dsl = slice(cuts[piece], cuts[piece + 1])
                            nc.sync.dma_start(xts[:, dsl],
                                              xT.ap()[:, 0, dsl, :])
                            for n in range(KH):
                                nc.sync.dma_start(wk_t[:, n, dsl],
                                                  wk.ap()[n][:, dsl])
                        load_tables(0, SC, consts=True)
                        hd = DT // 2
                        for half_ in range(2):
                            dsl = slice(half_ * hd, (half_ + 1) * hd)
                            for n in range(KH):
                                nc.sync.dma_start(wv_t[:, dsl, n, :],
                                                  wv.ap()[n][:, dsl])
                    else:
                        hdt = DT // 2
                        nc.sync.dma_start(xts[:, 0:hdt],
                                          xT.ap()[:, ch, 0:hdt, :])
                        nc.sync.dma_start(xts[:, hdt:DT],
                                          xT.ap()[:, ch, hdt:DT, :])
                        if ch == 1:
                            # Remaining table columns and wq_pre aren't
                            # needed until later; keep them out of the
                            # startup burst.
                            load_tables(SC, S)
                            nc.sync.dma_start(wq_pre[:], wq.ap()[0])
                    for kh in range(KH):
                        pk = psk.tile([128, SC], F32, tag="pk")
                        for di in range(DT):
                            nc.tensor.matmul(
                                pk[:],
                                wk_t[:, kh, di, :],
                                xts[:, di, :],
                                start=(di == 0), stop=(di == DT - 1))
                        rope(pk[:], k_sb[:, kh, ch * SC:(ch + 1) * SC], ch * SC,
                             SC, rpA, psw, "tbs")
                    for tl in range(SC // 128):
                        pv = psv.tile([128, KH * H], F32, tag="pv")
                        for di in range(DT):
                            nc.tensor.matmul(
                                pv[:],
                                xts[:, di, tl * 128:(tl + 1) * 128],
                                wv_t[:, di].rearrange(
                                    "p a b -> p (a b)"),
                                start=(di == 0), stop=(di == DT - 1))
                        tt = ch * (SC // 128) + tl
                        nc.vector.tensor_copy(v_sb[:, tt, :], pv[:])

            # ---- FUSED PASS: q projection + RoPE + attention ----
            # Per (head-group, s-tile): project q for HG heads straight
            # into SBUF, then run their causal attention.  Projection
            # matmuls of iteration i+1 overlap attention of iteration i.
            es = ExitStack()
            with es:
                es.enter_context(wqp_es.pop_all())
                es.enter_context(xtp_es.pop_all())
                wqp = es.enter_context(tc.tile_pool(name="wqp", bufs=1))
                rpB = es.enter_context(tc.tile_pool(name="ropeB", bufs=2))
                qjp = es.enter_context(tc.tile_pool(name="qj", bufs=5))
                wtp = es.enter_context(tc.tile_pool(name="wt", bufs=6))
                rcpp = es.enter_context(tc.tile_pool(name="rcp", bufs=2))
                onp = es.enter_context(tc.tile_pool(name="on", bufs=2))
                accp = es.enter_context(tc.tile_pool(name="accp", bufs=2))
                psq = es.enter_context(
                    tc.tile_pool(name="psqB", bufs=2, space="PSUM"))
                pss = es.enter_context(
                    tc.tile_pool(name="pss", bufs=3, space="PSUM"))
                pso = es.enter_context(
                    tc.tile_pool(name="pso", bufs=2, space="PSUM"))
                psd = es.enter_context(
                    tc.tile_pool(name="psd", bufs=1, space="PSUM"))

                # Block schedule: one block = (head-group, s-tile).  The
                # NEXT block's q-projection heads are interleaved between
                # the current block's attention heads so the PE always has
                # Act-independent work while exp drains at head tails.
                sched = [(hg, j) for hg in range(QH // HG)
                         for j in reversed(range(NJ))]
                NB = len(sched)
                qj_store = {}
                xts_store = {}

                def qproj_head(bi, hl, wqs):
                    hg, j = sched[bi]
                    if hl == 0:
                        qj_store[bi] = [
                            qjp.tile([128, ST], BF16, tag="qj", bufs=8,
                                     name="qj%d_%d" % (bi, hh))
                            for hh in range(HG)]
                        xts = xtsp.tile([128, DT, SC], BF16, tag="xts")
                        hdt = DT // 2
                        nc.sync.dma_start(xts[:, 0:hdt],
                                          xT.ap()[:, j, 0:hdt, :])
                        nc.sync.dma_start(xts[:, hdt:DT],
                                          xT.ap()[:, j, hdt:DT, :])
                        xts_store[bi] = xts
                    xts = xts_store[bi]
                    pq = psq.tile([128, SC], F32, tag="pq")
                    for di in range(DT):
                        nc.tensor.matmul(
                            pq[:], wqs[hl][:, di, :], xts[:, di, :],
                            start=(di == 0), stop=(di == DT - 1))
                    rope(pq[:], qj_store[bi][hl][:], j * ST, SC, rpB,
                         pss, "ps")

                # Deferred epilogue: head h's normalize (bcast matmul +
                # multiply + oda DMA) is issued at the START of head h+1 so
                # the PE never waits on the slow [1,ST] reciprocal.
                pending = []

                def flush_epilogue():
                    if not pending:
                        return
                    h, j, po, rcr = pending.pop()
                    pb = pss.tile([128, ST], F32, tag="ps", name="pb")
                    nc.tensor.matmul(pb[:], ones_row, rcr[:],
                                     start=True, stop=True)
                    # DVE cannot read two PSUM operands: stage the broadcast
                    # through SBUF on the (otherwise idle) Pool engine.
                    pbs = onp.tile([128, ST], BF16, tag="pbs", bufs=1)
                    nc.scalar.activation(pbs[:], pb[:], COPY)
                    nc.vector.tensor_tensor(o_sb[:, h, j * ST:(j + 1) * ST],
                                            po[:], pbs[:], MULT)

                def attn_head(h, qjt, j):
                    kh = h // G
                    po = pso.tile([128, ST], F32, tag="po")
                    pden = psd.tile([1, ST], F32, tag="pden")
                    KT = (j + 1) * TJ
                    # Diagonal tiles first: their mask->exp chains clear the
                    # DVE/Act queues early instead of serializing the head
                    # tail.  Off-diagonal tiles then stream mask-free.
                    # Columns left of the diagonal block are never computed:
                    # scores/exp/AV/acc all operate on [lo:ST) only.
                    order = [j * TJ + m for m in range(TJ)]
                    order += list(range(j * TJ))
                    # Depth-3 software pipeline: scores run ahead so the PE
                    # doesn't wait the ~600ns mask+exp latency per tile.
                    pend_av = []
                    acc = accp.tile([128, ST], BF16, tag="acc")
                    fired = [0]

                    def fire_av():
                        wt_, kt_, lo_ = pend_av.pop(0)
                        nc.tensor.matmul(
                            po[:, lo_:ST],
                            v_sb[:, kt_, kh * H:(kh + 1) * H], wt_[:, lo_:ST],
                            start=(fired[0] == 0), stop=(fired[0] == KT - 1))
                        fired[0] += 1

                    acc_defer = []

                    def flush_acc():
                        for first_, wt_, lo_ in acc_defer:
                            if first_:
                                nc.vector.tensor_copy(acc[:], wt_[:])
                            else:
                                nc.vector.tensor_tensor(acc[:, lo_:ST],
                                                        acc[:, lo_:ST],
                                                        wt_[:, lo_:ST], ADD)
                        del acc_defer[:]

                    for idx, kt in enumerate(order):
                        m = kt - j * TJ
                        lo = m * 128 if m > 0 else 0
                        ps = pss.tile([128, ST], F32, tag="ps")
                        nc.tensor.matmul(
                            ps[:, lo:ST], k_sb[:, kh, kt * 128:(kt + 1) * 128],
                            qjt[:, lo:ST], start=True, stop=True)
                        if len(pend_av) >= 3:
                            fire_av()
                        wtile = wtp.tile([128, ST], BF16, tag="wt")
                        if m >= 0:
                            nc.vector.tensor_tensor(
                                ps[:, lo:lo + 128], ps[:, lo:lo + 128],
                                tri_sb[:], ADD)
                        nc.scalar.activation(wtile[:, lo:ST], ps[:, lo:ST],
                                             EXP, scale=scale)
                        # Denominator: accumulate exp tiles on DVE (all-bf16
                        # hits the fast mode); one ones-matmul per head does
                        # the partition-dim sum, keeping 320 matmuls off PE.
                        # Diag-tile adds are DEFERRED: issued per-tile they
                        # would queue in front of the next diag tile's mask
                        # in the DVE FIFO (acc waits on exp) and serialize
                        # the whole mask->exp->acc chain.
                        if idx < TJ:
                            acc_defer.append((idx == 0, wtile, lo))
                        else:
                            flush_acc()
                            nc.vector.tensor_tensor(acc[:, lo:ST],
                                                    acc[:, lo:ST],
                                                    wtile[:, lo:ST], ADD)
                        pend_av.append((wtile, kt, lo))
                    flush_acc()

                    # The tail (remaining AVs + den + normalize chain) is
                    # returned as a closure so the caller can slot PE work
                    # (next block's q-projection) before it — the tail AVs
                    # then never wait on exp.
                    def tail():
                        while pend_av:
                            fire_av()
                        nc.tensor.matmul(pden[:], ones_col, acc[:],
                                         start=True, stop=True)
                        dns = rcpp.tile([1, ST], F32, tag="dns")
                        nc.vector.tensor_copy(dns[:], pden[:])
                        rcf = rcpp.tile([1, ST], F32, tag="rcf")
                        nc.vector.reciprocal_approx_fast(rcf[:], dns[:])
                        rcr = rcpp.tile([1, ST], BF16, tag="rcr")
                        nc.scalar.activation(rcr[:], rcf[:], COPY)
                        flush_epilogue()
                        pending.append((h, j, po, rcr))
                    return tail

                def load_wq(hg):
                    tiles = []
                    for hl in range(HG):
                        if hg == 0 and hl == 0:
                            tiles.append(wq_pre)
                            continue
                        pool_ = wqp0 if hl == 0 else wqp
                        wt_ = pool_.tile([128, DT, H], BF16,
                                         tag="wq%d" % hl,
                                         name="wq_%d_%d" % (hg, hl))
                        nc.sync.dma_start(wt_[:], wq.ap()[hg * HG + hl])
                        tiles.append(wt_)
                    return tiles

                NHG = QH // HG
                wqs_by_hg = {0: load_wq(0)}
                for bi in range(NB):
                    hg, j = sched[bi]
                    if bi % NJ == 0:
                        for hl in range(HG):
                            qproj_head(bi, hl, wqs_by_hg[hg])
                    if bi == NJ - 1 and NHG > 1:
                        wqs_by_hg[1] = load_wq(1)
                    nxt = bi + 1
                    interleave = nxt < NB and nxt % NJ != 0
                    if bi == NB - 1:
                        # Prefetch the first o-proj weight tile; its first
                        # four column groups (s-tiles finished blocks ago)
                        # interleave into this last block's attention in
                        # place of a next q-projection.
                        woe0 = wop.tile([128, QH, EW], BF16, tag="woe",
                                        name="woe_pre")
                        for h in range(QH):
                            nc.sync.dma_start(
                                woe0[:, h, :], wo.ap()[h, :, 0:EW])

                    def passC_group(st):
                        pc = psq.tile([128, EW], F32, tag="pq", name="pcp")
                        for h in range(QH):
                            nc.tensor.matmul(
                                pc[:],
                                o_sb[:, h, st * 128:(st + 1) * 128],
                                woe0[:, h, :],
                                start=(h == 0), stop=(h == QH - 1))
                        oc = onp.tile([128, EW], BF16, tag="oc0", bufs=2)
                        nc.vector.tensor_copy(oc[:], pc[:])
                        nc.scalar.dma_start(
                            o_out[st * 128:(st + 1) * 128, 0:EW], oc[:])

                    for hl in range(HG):
                        tail = attn_head(hg * HG + hl, qj_store[bi][hl], j)
                        if interleave:
                            nhg, _ = sched[nxt]
                            qproj_head(nxt, hl, wqs_by_hg[nhg])
                        elif bi == NB - 1:
                            passC_group(S // 128 - 1 - 3 * hl)
                            passC_group(S // 128 - 2 - 3 * hl)
                            passC_group(S // 128 - 3 - 3 * hl)
                        tail()
                    del qj_store[bi]
                flush_epilogue()

                # ---- PASS C: o projection, inside the pass-B scope so no
                # pool-release barrier sits between them.  The q-proj PSUM
                # pool (idle once the schedule drains) carries the psum
                # tiles; the prelude's staging tag carries the copies.
                NST = S // 128
                for e in range(NE):
                    if e == 0:
                        woe = woe0
                    else:
                        woe = wop.tile([128, QH, EW], BF16, tag="woe")
                        for h in range(QH):
                            nc.sync.dma_start(
                                woe[:, h, :],
                                wo.ap()[h, :, e * EW:(e + 1) * EW])
                    # Descending s-tiles: o_sb columns complete in that
                    # order (schedule runs j descending per head-group);
                    # the last twelve groups of e=0 ran inside pass B.
                    for st in reversed(range(NST - 12 if e == 0 else NST)):
                        pc = psq.tile([128, EW], F32, tag="pq", name="pc")
                        for h in range(QH):
                            nc.tensor.matmul(
                                pc[:],
                                o_sb[:, h, st * 128:(st + 1) * 128],
                                woe[:, h, :],
                                start=(h == 0), stop=(h == QH - 1))
                        oc = onp.tile([128, EW], BF16, tag="oc0", bufs=2)
                        nc.vector.tensor_copy(oc[:], pc[:])
                        nc.scalar.dma_start(
                            o_out[st * 128:(st + 1) * 128, e * EW:(e + 1) * EW],
                            oc[:])

    nc.compile()
    return nc


def _perm_matrix():
    P = np.zeros((128, 128), dtype=np.float32)
    P[np.arange(128), (np.arange(128) + 64) % 128] = 1.0
    return P


def make_tables(positions_b, S, H):
    """cos/sin tables in [128, S] layout with the sign fold for the swap
    trick (rows 0:63 -> +sin, 64:127 -> -sin), plus the triangular mask."""
    half = H // 2
    inv_freq = 1.0 / (ROPE_THETA ** (np.arange(half, dtype=np.float64) * 2.0 / H))
    ang = positions_b.astype(np.float64)[None, :] * inv_freq[:, None]  # [half, S]
    cos_h = np.cos(ang)
    sin_h = np.sin(ang)
    import ml_dtypes
    cos_t = np.concatenate([cos_h, cos_h], axis=0).astype(ml_dtypes.bfloat16)
    sin_t = np.concatenate([sin_h, -sin_h], axis=0).astype(ml_dtypes.bfloat16)
    idx = np.arange(128)
    tri = np.where(idx[:, None] <= idx[None, :], 0.0, NEG_BIG).astype(np.float32)
    return cos_t, sin_t, tri


def make_in_maps(x, positions, Wq, Wk, Wv, Wo, cfg):
    """Shard the full inputs into the 8 per-core input maps."""
    QH, KH = cfg["QH"], cfg["KH"]
    S, H = cfg["S"], cfg["H"]
    B = x.shape[0]
    groups = N_CORES // B
    tables = [make_tables(np.asarray(positions[b]), S, H) for b in range(B)]
    in_maps = []
    for c in range(N_CORES):
        b, g = divmod(c, groups)
        cos_t, sin_t, tri = tables[b]
        import ml_dtypes
        bf = ml_dtypes.bfloat16

        def pmaj(w):
            # [heads, D, H] -> [heads, 128, D/128, H] (partition-major so
            # each SBUF partition's data is one contiguous run)
            n, d, hh = w.shape
            return np.ascontiguousarray(
                w.reshape(n, d // 128, 128, hh).transpose(0, 2, 1, 3))

        xT_ds = np.asarray(x[b]).T.astype(bf)          # [D, S]
        d_, s_ = xT_ds.shape
        xc = np.ascontiguousarray(
            xT_ds.reshape(d_ // 128, 128, s_ // 512, 512)
            .transpose(1, 2, 0, 3))                    # [128, S/512, D/128, 512]
        in_maps.append({
            "xT": xc,
            "wq": pmaj(Wq[g * QH:(g + 1) * QH].astype(bf)),
            "wk": pmaj(Wk[g * KH:(g + 1) * KH].astype(bf)),
            "wv": pmaj(Wv[g * KH:(g + 1) * KH].astype(bf)),
            "wo": np.ascontiguousarray(Wo[g * QH:(g + 1) * QH].astype(bf)),
            "cos_t": cos_t,
            "sin_t": sin_t,
            "tri_t": tri,
            "ones_t": np.ones((128, 128), dtype=bf),
            "perm_t": _perm_matrix().astype(bf),
        })
    return in_maps


_NC_CACHE = {}


def _get_nc(cfg_key=None):
    cfg = FULL_CFG if cfg_key is None else cfg_key
    key = tuple(sorted(cfg.items()))
    if key not in _NC_CACHE:
        _NC_CACHE[key] = build_bass(cfg)
    return _NC_CACHE[key]


def run(x, positions, Wq, Wk, Wv, Wo, trace=False, trace_kwargs=None):
    cfg = FULL_CFG
    nc = _get_nc(cfg)
    in_maps = make_in_maps(np.asarray(x), np.asarray(positions),
                           np.asarray(Wq), np.asarray(Wk), np.asarray(Wv),
                           np.asarray(Wo), cfg)
    res = bass_utils.run_bass_kernel_spmd(
        nc, in_maps, list(range(N_CORES)), trace=trace,
        **(trace_kwargs or {}))
    B = np.asarray(x).shape[0]
    groups = N_CORES // B
    outs = []
    for b in range(B):
        acc = res.results[b * groups]["o_out"].astype(np.float64)
        for g in range(1, groups):
            acc += res.results[b * groups + g]["o_out"]
        outs.append(acc.astype(np.float32))
    return np.stack(outs, axis=0), res


def kernel(x, positions, Wq, Wk, Wv, Wo):
    out, _ = run(x, positions, Wq, Wk, Wv, Wo, trace=False)
    return out



# revision 80
# speedup vs baseline: 1.0141x; 1.0141x over previous
"""GQA causal-attention prefill kernel for 8 Trainium2 NeuronCores.

Reference computation (B=2, S=2048, D=4096, Q=32 q-heads, N=8 kv-heads,
H=128): QKV projection + RoPE + causal GQA attention + O projection.

Sharding: core c handles batch b = c//4 and kv-head pair g = c%4
(kv-heads 2g..2g+1, q-heads 8g..8g+7).  No collectives: each core
computes its partial o-projection (sum over its 8 q-heads) and the host
sums the four partials per batch at gather time (the "all-reduce").

Device-side layout strategy (per core):
  - x is fed pre-transposed ([D, S]) so projections contract over D with
    matmuls (stationary = weight / xT tile, moving N = s-chunk).
  - q, k are produced in [h, s] layout; v in [t, h] layout.
  - scores are computed transposed (S^T = K^T Q, psum [t, s]) so the
    softmax weights feed the AV matmul directly as the moving operand
    with t as the contraction partition - no transposes anywhere.
  - softmax denominators: exp tiles accumulate on DVE in bf16, then one
    ones-matmul per (head, s-tile) does the partition-dim sum; exp on
    ScalarE (fused 1/sqrt(H) scaling); causal masking via a triangular
    additive tile, with left-of-diagonal columns never computed at all.
  - attention inner loop is software-pipelined depth-3 (scores run ahead
    of AV) with diagonal tiles first; head tails (AV drain + normalize)
    are deferred behind the next block's q-projection so the PE never
    waits on exp latency.
  - RoPE in [h, s] layout: sign-folded sin table, rotate-half done as a
    TensorE matmul with a constant permutation matrix, then one add.
  - normalization happens in pass B: 1/den via fast-approx reciprocal,
    broadcast to 128 partitions with a rank-1 ones matmul, multiplied
    into a persistent SBUF o tile that pass C consumes directly.
  - all matmul operands are bf16 (PSUM accumulation stays fp32).
"""

import math
import sys

import numpy as np

for _p in ("/opt/trn_rl_repo", "/root/.axon_site/_ro/trn_rl_repo"):
    if _p not in sys.path:
        sys.path.append(_p)

import concourse.bacc as bacc
import concourse.mybir as mybir
import concourse.tile as tile
from concourse import bass_utils

dt = mybir.dt
F32 = dt.float32
F32R = dt.float32r
BF16 = dt.bfloat16
ADD = mybir.AluOpType.add
MULT = mybir.AluOpType.mult
EXP = mybir.ActivationFunctionType.Exp
COPY = mybir.ActivationFunctionType.Copy

# Full-problem config (per core after sharding).
FULL_CFG = dict(S=2048, D=4096, QH=8, KH=2, H=128, SC=512, ST=512, HG=4)
N_CORES = 8
ROPE_THETA = 10000.0
NEG_BIG = -1.0e30


def build_bass(cfg):
    S, D, QH, KH, H = cfg["S"], cfg["D"], cfg["QH"], cfg["KH"], cfg["H"]
    SC, ST, HG = cfg["SC"], cfg["ST"], cfg["HG"]
    assert H == 128 and D % 128 == 0 and S % SC == 0 and SC % 128 == 0
    assert S % ST == 0 and ST % 128 == 0 and QH % KH == 0 and QH % HG == 0
    DT = D // 128          # d-tiles (contraction tiles for projections)
    NCH = S // SC          # s-chunks for projections
    NJ = S // ST           # s-tiles for attention
    TJ = ST // 128         # 128-wide t-tiles per attention s-tile
    NT = S // 128          # total t-tiles
    G = QH // KH           # GQA group size
    EW = 512               # o-proj output tile width
    NE = D // EW
    scale = 1.0 / math.sqrt(H)

    from contextlib import ExitStack

    nc = bacc.Bacc("TRN2", target_bir_lowering=False, debug=False,
                   enable_asserts=False, num_devices=N_CORES)

    xT = nc.dram_tensor("xT", [128, S // 512, D // 128, 512], BF16,
                        kind="ExternalInput")
    wq = nc.dram_tensor("wq", [QH, 128, D // 128, H], BF16,
                        kind="ExternalInput")
    wk = nc.dram_tensor("wk", [KH, 128, D // 128, H], BF16,
                        kind="ExternalInput")
    wv = nc.dram_tensor("wv", [KH, 128, D // 128, H], BF16,
                        kind="ExternalInput")
    wo = nc.dram_tensor("wo", [QH, H, D], BF16, kind="ExternalInput")
    cos_d = nc.dram_tensor("cos_t", [128, S], BF16, kind="ExternalInput")
    sin_d = nc.dram_tensor("sin_t", [128, S], BF16, kind="ExternalInput")
    tri_d = nc.dram_tensor("tri_t", [128, 128], F32, kind="ExternalInput")
    ones_d = nc.dram_tensor("ones_t", [128, 128], BF16, kind="ExternalInput")
    perm_d = nc.dram_tensor("perm_t", [128, 128], BF16, kind="ExternalInput")
    o_out = nc.dram_tensor("o_out", [S, D], BF16, kind="ExternalOutput")


    with tile.TileContext(nc) as tc, \
         nc.allow_low_precision(reason="deliberate fp32r matmul pipeline"):
        with tc.tile_pool(name="persist", bufs=1) as persist, \
             tc.tile_pool(name="wop", bufs=2) as wop, \
             tc.tile_pool(name="drsc", bufs=1, space="DRAM") as dram:
            cos_sb = persist.tile([128, S], BF16)
            sin_sb = persist.tile([128, S], BF16)
            tri_sb = persist.tile([128, 128], F32)
            ones_sb = persist.tile([128, 128], BF16)
            perm_sb = persist.tile([128, 128], BF16)
            k_sb = persist.tile([128, KH, S], BF16)
            v_sb = persist.tile([128, NT, KH * H], BF16)
            o_sb = persist.tile([128, QH, S], BF16)
            ones_col = ones_sb[:, 0:1]
            ones_row = ones_sb[0:1, :]

            def load_tables(lo, hi, consts=False):
                # cos/sin split by column range so only the first chunk's
                # slice rides the startup DMA burst.
                nc.sync.dma_start(cos_sb[:, lo:hi], cos_d[:, lo:hi])
                nc.sync.dma_start(sin_sb[:, lo:hi], sin_d[:, lo:hi])
                if consts:
                    nc.sync.dma_start(tri_sb[:], tri_d[:, :])
                    nc.sync.dma_start(ones_sb[:], ones_d[:, :])
                    nc.sync.dma_start(perm_sb[:], perm_d[:, :])

            def rope(ps_tile, dst_ap, s0, W, rp, swp, swtag):
                """dst = rope(ps_tile) for s-range [s0, s0+W).

                The rotate-half partition swap runs on TensorE as a
                matmul with a constant permutation matrix (sign folded
                into the sin table), keeping the chain DMA-free."""
                ta = rp.tile([128, W], F32, tag="ta")
                tb = rp.tile([128, W], BF16, tag="tb")
                csl = cos_sb[:, s0:s0 + W]
                ssl = sin_sb[:, s0:s0 + W]
                nc.vector.tensor_tensor(ta[:], ps_tile, csl, MULT)
                nc.vector.tensor_tensor(tb[:], ps_tile, ssl, MULT)
                tbs = swp.tile([128, W], F32, tag=swtag)
                nc.tensor.matmul(tbs[:], perm_sb[:], tb[:],
                                 start=True, stop=True)
                nc.vector.tensor_tensor(dst_ap, ta[:], tbs[:], ADD)

            # wq head tiles live in their own pool opened before pass A
            # so the first q-weight load overlaps k/v projection.
            wqp_es = ExitStack()
            wqp0 = wqp_es.enter_context(tc.tile_pool(name="wqp0", bufs=1))
            wq_pre = wqp0.tile([128, DT, H], BF16, tag="wq0", name="wq_pre")
            xtp_es = ExitStack()
            xtsp = xtp_es.enter_context(tc.tile_pool(name="xts", bufs=2))

            # ---- PASS A: k and v projections (+ RoPE on k) ----
            with nc.named_scope("passA"), \
                 tc.tile_pool(name="wkv", bufs=1) as wkvp, \
                 tc.tile_pool(name="ropeA", bufs=2) as rpA, \
                 tc.tile_pool(name="pskA", bufs=3, space="PSUM") as psk, \
                 tc.tile_pool(name="psvA", bufs=3, space="PSUM") as psv, \
                 tc.tile_pool(name="pswA", bufs=2, space="PSUM") as psw:
                wk_t = wkvp.tile([128, KH, DT, H], BF16)
                wv_t = wkvp.tile([128, DT, KH, H], BF16)
                first_loads_done = False
                for ch in range(NCH):
                    xts = xtsp.tile([128, DT, SC], BF16, tag="xts")
                    if not first_loads_done:
                        first_loads_done = True
                        cuts = [0, 1, 4, 12, DT]
                        for piece in range(4):
                            dsl = slice(cuts[piece], cuts[piece + 1])
                            nc.sync.dma_start(xts[:, dsl],
                                              xT.ap()[:, 0, dsl, :])
                            for n in range(KH):
                                nc.sync.dma_start(wk_t[:, n, dsl],
                                                  wk.ap()[n][:, dsl])
                        load_tables(0, SC, consts=True)
                        hd = DT // 2
                        for half_ in range(2):
                            dsl = slice(half_ * hd, (half_ + 1) * hd)
                            for n in range(KH):
                                nc.sync.dma_start(wv_t[:, dsl, n, :],
                                                  wv.ap()[n][:, dsl])
                    else:
                        hdt = DT // 2
                        nc.sync.dma_start(xts[:, 0:hdt],
                                          xT.ap()[:, ch, 0:hdt, :])
                        nc.sync.dma_start(xts[:, hdt:DT],
                                          xT.ap()[:, ch, hdt:DT, :])
                        if ch == 1:
                            # Remaining table columns and wq_pre aren't
                            # needed until later; keep them out of the
                            # startup burst.
                            load_tables(SC, S)
                            nc.sync.dma_start(wq_pre[:], wq.ap()[0])
                    for kh in range(KH):
                        pk = psk.tile([128, SC], F32, tag="pk")
                        for di in range(DT):
                            nc.tensor.matmul(
                                pk[:],
                                wk_t[:, kh, di, :],
                                xts[:, di, :],
                                start=(di == 0), stop=(di == DT - 1))
                        rope(pk[:], k_sb[:, kh, ch * SC:(ch + 1) * SC], ch * SC,
                             SC, rpA, psw, "tbs")
                    for tl in range(SC // 128):
                        pv = psv.tile([128, KH * H], F32, tag="pv")
                        for di in range(DT):
                            nc.tensor.matmul(
                                pv[:],
                                xts[:, di, tl * 128:(tl + 1) * 128],
                                wv_t[:, di].rearrange(
                                    "p a b -> p (a b)"),
                                start=(di == 0), stop=(di == DT - 1))
                        tt = ch * (SC // 128) + tl
                        nc.vector.tensor_copy(v_sb[:, tt, :], pv[:])

            # ---- FUSED PASS: q projection + RoPE + attention ----
            # Per (head-group, s-tile): project q for HG heads straight
            # into SBUF, then run their causal attention.  Projection
            # matmuls of iteration i+1 overlap attention of iteration i.
            es = ExitStack()
            with es:
                es.enter_context(wqp_es.pop_all())
                es.enter_context(xtp_es.pop_all())
                wqp = es.enter_context(tc.tile_pool(name="wqp", bufs=1))
                rpB = es.enter_context(tc.tile_pool(name="ropeB", bufs=2))
                qjp = es.enter_context(tc.tile_pool(name="qj", bufs=5))
                wtp = es.enter_context(tc.tile_pool(name="wt", bufs=6))
                rcpp = es.enter_context(tc.tile_pool(name="rcp", bufs=2))
                onp = es.enter_context(tc.tile_pool(name="on", bufs=2))
                accp = es.enter_context(tc.tile_pool(name="accp", bufs=2))
                psq = es.enter_context(
                    tc.tile_pool(name="psqB", bufs=2, space="PSUM"))
                pss = es.enter_context(
                    tc.tile_pool(name="pss", bufs=3, space="PSUM"))
                pso = es.enter_context(
                    tc.tile_pool(name="pso", bufs=2, space="PSUM"))
                psd = es.enter_context(
                    tc.tile_pool(name="psd", bufs=1, space="PSUM"))

                # Block schedule: one block = (head-group, s-tile).  The
                # NEXT block's q-projection heads are interleaved between
                # the current block's attention heads so the PE always has
                # Act-independent work while exp drains at head tails.
                sched = [(hg, j) for hg in range(QH // HG)
                         for j in reversed(range(NJ))]
                NB = len(sched)
                qj_store = {}
                xts_store = {}

                def qproj_head(bi, hl, wqs):
                    hg, j = sched[bi]
                    if hl == 0:
                        qj_store[bi] = [
                            qjp.tile([128, ST], BF16, tag="qj", bufs=8,
                                     name="qj%d_%d" % (bi, hh))
                            for hh in range(HG)]
                        xts = xtsp.tile([128, DT, SC], BF16, tag="xts")
                        hdt = DT // 2
                        nc.sync.dma_start(xts[:, 0:hdt],
                                          xT.ap()[:, j, 0:hdt, :])
                        nc.sync.dma_start(xts[:, hdt:DT],
                                          xT.ap()[:, j, hdt:DT, :])
                        xts_store[bi] = xts
                    xts = xts_store[bi]
                    pq = psq.tile([128, SC], F32, tag="pq")
                    for di in range(DT):
                        nc.tensor.matmul(
                            pq[:], wqs[hl][:, di, :], xts[:, di, :],
                            start=(di == 0), stop=(di == DT - 1))
                    rope(pq[:], qj_store[bi][hl][:], j * ST, SC, rpB,
                         pss, "ps")

                # Deferred epilogue: head h's normalize (bcast matmul +
                # multiply + oda DMA) is issued at the START of head h+1 so
                # the PE never waits on the slow [1,ST] reciprocal.
                pending = []

                def flush_epilogue():
                    if not pending:
                        return
                    h, j, po, rcr = pending.pop()
                    pb = pss.tile([128, ST], F32, tag="ps", name="pb")
                    nc.tensor.matmul(pb[:], ones_row, rcr[:],
                                     start=True, stop=True)
                    # DVE cannot read two PSUM operands: stage the broadcast
                    # through SBUF on the (otherwise idle) Pool engine.
                    pbs = onp.tile([128, ST], BF16, tag="pbs", bufs=1)
                    nc.scalar.activation(pbs[:], pb[:], COPY)
                    nc.vector.tensor_tensor(o_sb[:, h, j * ST:(j + 1) * ST],
                                            po[:], pbs[:], MULT)

                def attn_head(h, qjt, j):
                    kh = h // G
                    po = pso.tile([128, ST], F32, tag="po")
                    pden = psd.tile([1, ST], F32, tag="pden")
                    KT = (j + 1) * TJ
                    # Diagonal tiles first: their mask->exp chains clear the
                    # DVE/Act queues early instead of serializing the head
                    # tail.  Off-diagonal tiles then stream mask-free.
                    # Columns left of the diagonal block are never computed:
                    # scores/exp/AV/acc all operate on [lo:ST) only.
                    order = [j * TJ + m for m in range(TJ)]
                    order += list(range(j * TJ))
                    # Depth-3 software pipeline: scores run ahead so the PE
                    # doesn't wait the ~600ns mask+exp latency per tile.
                    pend_av = []
                    acc = accp.tile([128, ST], BF16, tag="acc")
                    fired = [0]

                    def fire_av():
                        wt_, kt_, lo_ = pend_av.pop(0)
                        nc.tensor.matmul(
                            po[:, lo_:ST],
                            v_sb[:, kt_, kh * H:(kh + 1) * H], wt_[:, lo_:ST],
                            start=(fired[0] == 0), stop=(fired[0] == KT - 1))
                        fired[0] += 1

                    acc_defer = []

                    def flush_acc():
                        for first_, wt_, lo_ in acc_defer:
                            if first_:
                                nc.vector.tensor_copy(acc[:], wt_[:])
                            else:
                                nc.vector.tensor_tensor(acc[:, lo_:ST],
                                                        acc[:, lo_:ST],
                                                        wt_[:, lo_:ST], ADD)
                        del acc_defer[:]

                    for idx, kt in enumerate(order):
                        m = kt - j * TJ
                        lo = m * 128 if m > 0 else 0
                        ps = pss.tile([128, ST], F32, tag="ps")
                        nc.tensor.matmul(
                            ps[:, lo:ST], k_sb[:, kh, kt * 128:(kt + 1) * 128],
                            qjt[:, lo:ST], start=True, stop=True)
                        if len(pend_av) >= 3:
                            fire_av()
                        wtile = wtp.tile([128, ST], BF16, tag="wt")
                        if m >= 0:
                            nc.vector.tensor_tensor(
                                ps[:, lo:lo + 128], ps[:, lo:lo + 128],
                                tri_sb[:], ADD)
                        nc.scalar.activation(wtile[:, lo:ST], ps[:, lo:ST],
                                             EXP, scale=scale)
                        # Denominator: accumulate exp tiles on DVE (all-bf16
                        # hits the fast mode); one ones-matmul per head does
                        # the partition-dim sum, keeping 320 matmuls off PE.
                        # Diag-tile adds are DEFERRED: issued per-tile they
                        # would queue in front of the next diag tile's mask
                        # in the DVE FIFO (acc waits on exp) and serialize
                        # the whole mask->exp->acc chain.
                        if idx < TJ:
                            acc_defer.append((idx == 0, wtile, lo))
                        else:
                            flush_acc()
                            nc.vector.tensor_tensor(acc[:, lo:ST],
                                                    acc[:, lo:ST],
                                                    wtile[:, lo:ST], ADD)
                        pend_av.append((wtile, kt, lo))
                    flush_acc()

                    # The tail (remaining AVs + den + normalize chain) is
                    # returned as a closure so the caller can slot PE work
                    # (next block's q-projection) before it — the tail AVs
                    # then never wait on exp.
                    def tail():
                        while pend_av:
                            fire_av()
                        nc.tensor.matmul(pden[:], ones_col, acc[:],
                                         start=True, stop=True)
                        dns = rcpp.tile([1, ST], F32, tag="dns")
                        nc.vector.tensor_copy(dns[:], pden[:])
                        rcf = rcpp.tile([1, ST], F32, tag="rcf")
                        nc.vector.reciprocal_approx_fast(rcf[:], dns[:])
                        rcr = rcpp.tile([1, ST], BF16, tag="rcr")
                        nc.scalar.activation(rcr[:], rcf[:], COPY)
                        flush_epilogue()
                        pending.append((h, j, po, rcr))
                    return tail

                def load_wq(hg):
                    tiles = []
                    for hl in range(HG):
                        if hg == 0 and hl == 0:
                            tiles.append(wq_pre)
                            continue
                        pool_ = wqp0 if hl == 0 else wqp
                        wt_ = pool_.tile([128, DT, H], BF16,
                                         tag="wq%d" % hl,
                                         name="wq_%d_%d" % (hg, hl))
                        nc.sync.dma_start(wt_[:], wq.ap()[hg * HG + hl])
                        tiles.append(wt_)
                    return tiles

                NHG = QH // HG
                wqs_by_hg = {0: load_wq(0)}
                for bi in range(NB):
                    hg, j = sched[bi]
                    if bi % NJ == 0:
                        for hl in range(HG):
                            qproj_head(bi, hl, wqs_by_hg[hg])
                    if bi == NJ - 1 and NHG > 1:
                        wqs_by_hg[1] = load_wq(1)
                    nxt = bi + 1
                    interleave = nxt < NB and nxt % NJ != 0
                    if bi == NB - 1:
                        # Prefetch the first o-proj weight tile; its first
                        # four column groups (s-tiles finished blocks ago)
                        # interleave into this last block's attention in
                        # place of a next q-projection.
                        woe0 = wop.tile([128, QH, EW], BF16, tag="woe",
                                        name="woe_pre")
                        for h in range(QH):
                            nc.sync.dma_start(
                                woe0[:, h, :], wo.ap()[h, :, 0:EW])

                    def passC_group(st):
                        pc = psq.tile([128, EW], F32, tag="pq", name="pcp")
                        for h in range(QH):
                            nc.tensor.matmul(
                                pc[:],
                                o_sb[:, h, st * 128:(st + 1) * 128],
                                woe0[:, h, :],
                                start=(h == 0), stop=(h == QH - 1))
                        oc = onp.tile([128, EW], BF16, tag="oc0", bufs=2)
                        nc.vector.tensor_copy(oc[:], pc[:])
                        nc.scalar.dma_start(
                            o_out[st * 128:(st + 1) * 128, 0:EW], oc[:])

                    for hl in range(HG):
                        tail = attn_head(hg * HG + hl, qj_store[bi][hl], j)
                        if interleave:
                            nhg, _ = sched[nxt]
                            qproj_head(nxt, hl, wqs_by_hg[nhg])
                        elif bi == NB - 1:
                            passC_group(S // 128 - 1 - 3 * hl)
                            passC_group(S // 128 - 2 - 3 * hl)
                            passC_group(S // 128 - 3 - 3 * hl)
                        tail()
                    del qj_store[bi]
                flush_epilogue()

            # ---- PASS C: o projection (partial over this core's heads) ----
            # Attention outputs are already normalized in SBUF (o_sb).
            with nc.named_scope("passC"), \
                 tc.tile_pool(name="ocp", bufs=3) as ocp, \
                 tc.tile_pool(name="psc", bufs=4, space="PSUM") as psc:
                NST = S // 128
                for e in range(NE):
                    if e == 0:
                        woe = woe0
                    else:
                        woe = wop.tile([128, QH, EW], BF16, tag="woe")
                        for h in range(QH):
                            nc.sync.dma_start(
                                woe[:, h, :],
                                wo.ap()[h, :, e * EW:(e + 1) * EW])
                    # Descending s-tiles: o_sb columns complete in that
                    # order (schedule runs j descending per head-group);
                    # the last twelve groups of e=0 ran inside pass B.
                    for st in reversed(range(NST - 12 if e == 0 else NST)):
                        pc = psc.tile([128, EW], F32, tag="pc")
                        for h in range(QH):
                            nc.tensor.matmul(
                                pc[:],
                                o_sb[:, h, st * 128:(st + 1) * 128],
                                woe[:, h, :],
                                start=(h == 0), stop=(h == QH - 1))
                        oc = ocp.tile([128, EW], BF16, tag="oc")
                        nc.vector.tensor_copy(oc[:], pc[:])
                        nc.scalar.dma_start(
                            o_out[st * 128:(st + 1) * 128, e * EW:(e + 1) * EW],
                            oc[:])

    nc.compile()
    return nc


def _perm_matrix():
    P = np.zeros((128, 128), dtype=np.float32)
    P[np.arange(128), (np.arange(128) + 64) % 128] = 1.0
    return P


def make_tables(positions_b, S, H):
    """cos/sin tables in [128, S] layout with the sign fold for the swap
    trick (rows 0:63 -> +sin, 64:127 -> -sin), plus the triangular mask."""
    half = H // 2
    inv_freq = 1.0 / (ROPE_THETA ** (np.arange(half, dtype=np.float64) * 2.0 / H))
    ang = positions_b.astype(np.float64)[None, :] * inv_freq[:, None]  # [half, S]
    cos_h = np.cos(ang)
    sin_h = np.sin(ang)
    import ml_dtypes
    cos_t = np.concatenate([cos_h, cos_h], axis=0).astype(ml_dtypes.bfloat16)
    sin_t = np.concatenate([sin_h, -sin_h], axis=0).astype(ml_dtypes.bfloat16)
    idx = np.arange(128)
    tri = np.where(idx[:, None] <= idx[None, :], 0.0, NEG_BIG).astype(np.float32)
    return cos_t, sin_t, tri


def make_in_maps(x, positions, Wq, Wk, Wv, Wo, cfg):
    """Shard the full inputs into the 8 per-core input maps."""
    QH, KH = cfg["QH"], cfg["KH"]
    S, H = cfg["S"], cfg["H"]
    B = x.shape[0]
    groups = N_CORES // B
    tables = [make_tables(np.asarray(positions[b]), S, H) for b in range(B)]
    in_maps = []
    for c in range(N_CORES):
        b, g = divmod(c, groups)
        cos_t, sin_t, tri = tables[b]
        import ml_dtypes
        bf = ml_dtypes.bfloat16

        def pmaj(w):
            # [heads, D, H] -> [heads, 128, D/128, H] (partition-major so
            # each SBUF partition's data is one contiguous run)
            n, d, hh = w.shape
            return np.ascontiguousarray(
                w.reshape(n, d // 128, 128, hh).transpose(0, 2, 1, 3))

        xT_ds = np.asarray(x[b]).T.astype(bf)          # [D, S]
        d_, s_ = xT_ds.shape
        xc = np.ascontiguousarray(
            xT_ds.reshape(d_ // 128, 128, s_ // 512, 512)
            .transpose(1, 2, 0, 3))                    # [128, S/512, D/128, 512]
        in_maps.append({
            "xT": xc,
            "wq": pmaj(Wq[g * QH:(g + 1) * QH].astype(bf)),
            "wk": pmaj(Wk[g * KH:(g + 1) * KH].astype(bf)),
            "wv": pmaj(Wv[g * KH:(g + 1) * KH].astype(bf)),
            "wo": np.ascontiguousarray(Wo[g * QH:(g + 1) * QH].astype(bf)),
            "cos_t": cos_t,
            "sin_t": sin_t,
            "tri_t": tri,
            "ones_t": np.ones((128, 128), dtype=bf),
            "perm_t": _perm_matrix().astype(bf),
        })
    return in_maps


_NC_CACHE = {}


def _get_nc(cfg_key=None):
    cfg = FULL_CFG if cfg_key is None else cfg_key
    key = tuple(sorted(cfg.items()))
    if key not in _NC_CACHE:
        _NC_CACHE[key] = build_bass(cfg)
    return _NC_CACHE[key]


def run(x, positions, Wq, Wk, Wv, Wo, trace=False, trace_kwargs=None):
    cfg = FULL_CFG
    nc = _get_nc(cfg)
    in_maps = make_in_maps(np.asarray(x), np.asarray(positions),
                           np.asarray(Wq), np.asarray(Wk), np.asarray(Wv),
                           np.asarray(Wo), cfg)
    res = bass_utils.run_bass_kernel_spmd(
        nc, in_maps, list(range(N_CORES)), trace=trace,
        **(trace_kwargs or {}))
    B = np.asarray(x).shape[0]
    groups = N_CORES // B
    outs = []
    for b in range(B):
        acc = res.results[b * groups]["o_out"].astype(np.float64)
        for g in range(1, groups):
            acc += res.results[b * groups + g]["o_out"]
        outs.append(acc.astype(np.float32))
    return np.stack(outs, axis=0), res


def kernel(x, positions, Wq, Wk, Wv, Wo):
    out, _ = run(x, positions, Wq, Wk, Wv, Wo, trace=False)
    return out



# revision 81
# speedup vs baseline: 1.0156x; 1.0015x over previous
"""GQA causal-attention prefill kernel for 8 Trainium2 NeuronCores.

Reference computation (B=2, S=2048, D=4096, Q=32 q-heads, N=8 kv-heads,
H=128): QKV projection + RoPE + causal GQA attention + O projection.

Sharding: core c handles batch b = c//4 and kv-head pair g = c%4
(kv-heads 2g..2g+1, q-heads 8g..8g+7).  No collectives: each core
computes its partial o-projection (sum over its 8 q-heads) and the host
sums the four partials per batch at gather time (the "all-reduce").

Device-side layout strategy (per core):
  - x is fed pre-transposed ([D, S]) so projections contract over D with
    matmuls (stationary = weight / xT tile, moving N = s-chunk).
  - q, k are produced in [h, s] layout; v in [t, h] layout.
  - scores are computed transposed (S^T = K^T Q, psum [t, s]) so the
    softmax weights feed the AV matmul directly as the moving operand
    with t as the contraction partition - no transposes anywhere.
  - softmax denominators: exp tiles accumulate on DVE in bf16, then one
    ones-matmul per (head, s-tile) does the partition-dim sum; exp on
    ScalarE (fused 1/sqrt(H) scaling); causal masking via a triangular
    additive tile, with left-of-diagonal columns never computed at all.
  - attention inner loop is software-pipelined depth-3 (scores run ahead
    of AV) with diagonal tiles first; head tails (AV drain + normalize)
    are deferred behind the next block's q-projection so the PE never
    waits on exp latency.
  - RoPE in [h, s] layout: sign-folded sin table, rotate-half done as a
    TensorE matmul with a constant permutation matrix, then one add.
  - normalization happens in pass B: 1/den via fast-approx reciprocal,
    broadcast to 128 partitions with a rank-1 ones matmul, multiplied
    into a persistent SBUF o tile that pass C consumes directly.
  - all matmul operands are bf16 (PSUM accumulation stays fp32).
"""

import math
import sys

import numpy as np

for _p in ("/opt/trn_rl_repo", "/root/.axon_site/_ro/trn_rl_repo"):
    if _p not in sys.path:
        sys.path.append(_p)

import concourse.bacc as bacc
import concourse.mybir as mybir
import concourse.tile as tile
from concourse import bass_utils

dt = mybir.dt
F32 = dt.float32
F32R = dt.float32r
BF16 = dt.bfloat16
ADD = mybir.AluOpType.add
MULT = mybir.AluOpType.mult
EXP = mybir.ActivationFunctionType.Exp
COPY = mybir.ActivationFunctionType.Copy

# Full-problem config (per core after sharding).
FULL_CFG = dict(S=2048, D=4096, QH=8, KH=2, H=128, SC=512, ST=512, HG=4)
N_CORES = 8
ROPE_THETA = 10000.0
NEG_BIG = -1.0e30


def build_bass(cfg):
    S, D, QH, KH, H = cfg["S"], cfg["D"], cfg["QH"], cfg["KH"], cfg["H"]
    SC, ST, HG = cfg["SC"], cfg["ST"], cfg["HG"]
    assert H == 128 and D % 128 == 0 and S % SC == 0 and SC % 128 == 0
    assert S % ST == 0 and ST % 128 == 0 and QH % KH == 0 and QH % HG == 0
    DT = D // 128          # d-tiles (contraction tiles for projections)
    NCH = S // SC          # s-chunks for projections
    NJ = S // ST           # s-tiles for attention
    TJ = ST // 128         # 128-wide t-tiles per attention s-tile
    NT = S // 128          # total t-tiles
    G = QH // KH           # GQA group size
    EW = 512               # o-proj output tile width
    NE = D // EW
    scale = 1.0 / math.sqrt(H)

    from contextlib import ExitStack

    nc = bacc.Bacc("TRN2", target_bir_lowering=False, debug=False,
                   enable_asserts=False, num_devices=N_CORES)

    xT = nc.dram_tensor("xT", [128, S // 512, D // 128, 512], BF16,
                        kind="ExternalInput")
    wq = nc.dram_tensor("wq", [QH, 128, D // 128, H], BF16,
                        kind="ExternalInput")
    wk = nc.dram_tensor("wk", [KH, 128, D // 128, H], BF16,
                        kind="ExternalInput")
    wv = nc.dram_tensor("wv", [KH, 128, D // 128, H], BF16,
                        kind="ExternalInput")
    wo = nc.dram_tensor("wo", [QH, H, D], BF16, kind="ExternalInput")
    cos_d = nc.dram_tensor("cos_t", [128, S], BF16, kind="ExternalInput")
    sin_d = nc.dram_tensor("sin_t", [128, S], BF16, kind="ExternalInput")
    tri_d = nc.dram_tensor("tri_t", [128, 128], F32, kind="ExternalInput")
    ones_d = nc.dram_tensor("ones_t", [128, 128], BF16, kind="ExternalInput")
    perm_d = nc.dram_tensor("perm_t", [128, 128], BF16, kind="ExternalInput")
    o_out = nc.dram_tensor("o_out", [S, D], BF16, kind="ExternalOutput")


    with tile.TileContext(nc) as tc, \
         nc.allow_low_precision(reason="deliberate fp32r matmul pipeline"):
        with tc.tile_pool(name="persist", bufs=1) as persist, \
             tc.tile_pool(name="wop", bufs=2) as wop, \
             tc.tile_pool(name="drsc", bufs=1, space="DRAM") as dram:
            cos_sb = persist.tile([128, S], BF16)
            sin_sb = persist.tile([128, S], BF16)
            tri_sb = persist.tile([128, 128], F32)
            ones_sb = persist.tile([128, 128], BF16)
            perm_sb = persist.tile([128, 128], BF16)
            k_sb = persist.tile([128, KH, S], BF16)
            v_sb = persist.tile([128, NT, KH * H], BF16)
            o_sb = persist.tile([128, QH, S], BF16)
            ones_col = ones_sb[:, 0:1]
            ones_row = ones_sb[0:1, :]

            def load_tables(lo, hi, consts=False):
                # cos/sin split by column range so only the first chunk's
                # slice rides the startup DMA burst.
                nc.sync.dma_start(cos_sb[:, lo:hi], cos_d[:, lo:hi])
                nc.sync.dma_start(sin_sb[:, lo:hi], sin_d[:, lo:hi])
                if consts:
                    nc.sync.dma_start(tri_sb[:], tri_d[:, :])
                    nc.sync.dma_start(ones_sb[:], ones_d[:, :])
                    nc.sync.dma_start(perm_sb[:], perm_d[:, :])

            def rope(ps_tile, dst_ap, s0, W, rp, swp, swtag):
                """dst = rope(ps_tile) for s-range [s0, s0+W).

                The rotate-half partition swap runs on TensorE as a
                matmul with a constant permutation matrix (sign folded
                into the sin table), keeping the chain DMA-free."""
                ta = rp.tile([128, W], F32, tag="ta")
                tb = rp.tile([128, W], BF16, tag="tb")
                csl = cos_sb[:, s0:s0 + W]
                ssl = sin_sb[:, s0:s0 + W]
                nc.vector.tensor_tensor(ta[:], ps_tile, csl, MULT)
                nc.vector.tensor_tensor(tb[:], ps_tile, ssl, MULT)
                tbs = swp.tile([128, W], F32, tag=swtag)
                nc.tensor.matmul(tbs[:], perm_sb[:], tb[:],
                                 start=True, stop=True)
                nc.vector.tensor_tensor(dst_ap, ta[:], tbs[:], ADD)

            # wq head tiles live in their own pool opened before pass A
            # so the first q-weight load overlaps k/v projection.
            wqp_es = ExitStack()
            wqp0 = wqp_es.enter_context(tc.tile_pool(name="wqp0", bufs=1))
            wq_pre = wqp0.tile([128, DT, H], BF16, tag="wq0", name="wq_pre")
            xtp_es = ExitStack()
            xtsp = xtp_es.enter_context(tc.tile_pool(name="xts", bufs=2))

            # ---- PASS A: k and v projections (+ RoPE on k) ----
            with nc.named_scope("passA"), \
                 tc.tile_pool(name="wkv", bufs=1) as wkvp, \
                 tc.tile_pool(name="ropeA", bufs=2) as rpA, \
                 tc.tile_pool(name="pskA", bufs=3, space="PSUM") as psk, \
                 tc.tile_pool(name="psvA", bufs=3, space="PSUM") as psv, \
                 tc.tile_pool(name="pswA", bufs=2, space="PSUM") as psw:
                wk_t = wkvp.tile([128, KH, DT, H], BF16)
                wv_t = wkvp.tile([128, DT, KH, H], BF16)
                # Warm the PE out of its cold pstate during the initial DMA
                # wait: a dozen dummy matmuls on a memset scratch tile ramp
                # the clock so the first real matmuls run at full speed.
                # They complete before chunk 0's data lands.
                warm = wkvp.tile([128, 512], BF16, name="warm")
                nc.vector.memset(warm[:], 0)
                for wg in range(10):
                    pw = psk.tile([128, SC], F32, tag="pk",
                                  name="warm%d" % wg)
                    nc.tensor.matmul(pw[:], warm[:, 0:128], warm[:],
                                     start=True, stop=True)
                first_loads_done = False
                for ch in range(NCH):
                    xts = xtsp.tile([128, DT, SC], BF16, tag="xts")
                    if not first_loads_done:
                        first_loads_done = True
                        cuts = [0, 1, 4, 12, DT]
                        for piece in range(4):
                            dsl = slice(cuts[piece], cuts[piece + 1])
                            nc.sync.dma_start(xts[:, dsl],
                                              xT.ap()[:, 0, dsl, :])
                            for n in range(KH):
                                nc.sync.dma_start(wk_t[:, n, dsl],
                                                  wk.ap()[n][:, dsl])
                        load_tables(0, SC, consts=True)
                        hd = DT // 2
                        for half_ in range(2):
                            dsl = slice(half_ * hd, (half_ + 1) * hd)
                            for n in range(KH):
                                nc.sync.dma_start(wv_t[:, dsl, n, :],
                                                  wv.ap()[n][:, dsl])
                    else:
                        hdt = DT // 2
                        nc.sync.dma_start(xts[:, 0:hdt],
                                          xT.ap()[:, ch, 0:hdt, :])
                        nc.sync.dma_start(xts[:, hdt:DT],
                                          xT.ap()[:, ch, hdt:DT, :])
                        if ch == 1:
                            # Remaining table columns and wq_pre aren't
                            # needed until later; keep them out of the
                            # startup burst.
                            load_tables(SC, S)
                            nc.sync.dma_start(wq_pre[:], wq.ap()[0])
                    for kh in range(KH):
                        pk = psk.tile([128, SC], F32, tag="pk")
                        for di in range(DT):
                            nc.tensor.matmul(
                                pk[:],
                                wk_t[:, kh, di, :],
                                xts[:, di, :],
                                start=(di == 0), stop=(di == DT - 1))
                        rope(pk[:], k_sb[:, kh, ch * SC:(ch + 1) * SC], ch * SC,
                             SC, rpA, psw, "tbs")
                    for tl in range(SC // 128):
                        pv = psv.tile([128, KH * H], F32, tag="pv")
                        for di in range(DT):
                            nc.tensor.matmul(
                                pv[:],
                                xts[:, di, tl * 128:(tl + 1) * 128],
                                wv_t[:, di].rearrange(
                                    "p a b -> p (a b)"),
                                start=(di == 0), stop=(di == DT - 1))
                        tt = ch * (SC // 128) + tl
                        nc.vector.tensor_copy(v_sb[:, tt, :], pv[:])

            # ---- FUSED PASS: q projection + RoPE + attention ----
            # Per (head-group, s-tile): project q for HG heads straight
            # into SBUF, then run their causal attention.  Projection
            # matmuls of iteration i+1 overlap attention of iteration i.
            es = ExitStack()
            with es:
                es.enter_context(wqp_es.pop_all())
                es.enter_context(xtp_es.pop_all())
                wqp = es.enter_context(tc.tile_pool(name="wqp", bufs=1))
                rpB = es.enter_context(tc.tile_pool(name="ropeB", bufs=2))
                qjp = es.enter_context(tc.tile_pool(name="qj", bufs=5))
                wtp = es.enter_context(tc.tile_pool(name="wt", bufs=6))
                rcpp = es.enter_context(tc.tile_pool(name="rcp", bufs=2))
                onp = es.enter_context(tc.tile_pool(name="on", bufs=2))
                accp = es.enter_context(tc.tile_pool(name="accp", bufs=2))
                psq = es.enter_context(
                    tc.tile_pool(name="psqB", bufs=2, space="PSUM"))
                pss = es.enter_context(
                    tc.tile_pool(name="pss", bufs=3, space="PSUM"))
                pso = es.enter_context(
                    tc.tile_pool(name="pso", bufs=2, space="PSUM"))
                psd = es.enter_context(
                    tc.tile_pool(name="psd", bufs=1, space="PSUM"))

                # Block schedule: one block = (head-group, s-tile).  The
                # NEXT block's q-projection heads are interleaved between
                # the current block's attention heads so the PE always has
                # Act-independent work while exp drains at head tails.
                sched = [(hg, j) for hg in range(QH // HG)
                         for j in reversed(range(NJ))]
                NB = len(sched)
                qj_store = {}
                xts_store = {}

                def qproj_head(bi, hl, wqs):
                    hg, j = sched[bi]
                    if hl == 0:
                        qj_store[bi] = [
                            qjp.tile([128, ST], BF16, tag="qj", bufs=8,
                                     name="qj%d_%d" % (bi, hh))
                            for hh in range(HG)]
                        xts = xtsp.tile([128, DT, SC], BF16, tag="xts")
                        hdt = DT // 2
                        nc.sync.dma_start(xts[:, 0:hdt],
                                          xT.ap()[:, j, 0:hdt, :])
                        nc.sync.dma_start(xts[:, hdt:DT],
                                          xT.ap()[:, j, hdt:DT, :])
                        xts_store[bi] = xts
                    xts = xts_store[bi]
                    pq = psq.tile([128, SC], F32, tag="pq")
                    for di in range(DT):
                        nc.tensor.matmul(
                            pq[:], wqs[hl][:, di, :], xts[:, di, :],
                            start=(di == 0), stop=(di == DT - 1))
                    rope(pq[:], qj_store[bi][hl][:], j * ST, SC, rpB,
                         pss, "ps")

                # Deferred epilogue: head h's normalize (bcast matmul +
                # multiply + oda DMA) is issued at the START of head h+1 so
                # the PE never waits on the slow [1,ST] reciprocal.
                pending = []

                def flush_epilogue():
                    if not pending:
                        return
                    h, j, po, rcr = pending.pop()
                    pb = pss.tile([128, ST], F32, tag="ps", name="pb")
                    nc.tensor.matmul(pb[:], ones_row, rcr[:],
                                     start=True, stop=True)
                    # DVE cannot read two PSUM operands: stage the broadcast
                    # through SBUF on the (otherwise idle) Pool engine.
                    pbs = onp.tile([128, ST], BF16, tag="pbs", bufs=1)
                    nc.scalar.activation(pbs[:], pb[:], COPY)
                    nc.vector.tensor_tensor(o_sb[:, h, j * ST:(j + 1) * ST],
                                            po[:], pbs[:], MULT)

                def attn_head(h, qjt, j):
                    kh = h // G
                    po = pso.tile([128, ST], F32, tag="po")
                    pden = psd.tile([1, ST], F32, tag="pden")
                    KT = (j + 1) * TJ
                    # Diagonal tiles first: their mask->exp chains clear the
                    # DVE/Act queues early instead of serializing the head
                    # tail.  Off-diagonal tiles then stream mask-free.
                    # Columns left of the diagonal block are never computed:
                    # scores/exp/AV/acc all operate on [lo:ST) only.
                    order = [j * TJ + m for m in range(TJ)]
                    order += list(range(j * TJ))
                    # Depth-3 software pipeline: scores run ahead so the PE
                    # doesn't wait the ~600ns mask+exp latency per tile.
                    pend_av = []
                    acc = accp.tile([128, ST], BF16, tag="acc")
                    fired = [0]

                    def fire_av():
                        wt_, kt_, lo_ = pend_av.pop(0)
                        nc.tensor.matmul(
                            po[:, lo_:ST],
                            v_sb[:, kt_, kh * H:(kh + 1) * H], wt_[:, lo_:ST],
                            start=(fired[0] == 0), stop=(fired[0] == KT - 1))
                        fired[0] += 1

                    acc_defer = []

                    def flush_acc():
                        for first_, wt_, lo_ in acc_defer:
                            if first_:
                                nc.vector.tensor_copy(acc[:], wt_[:])
                            else:
                                nc.vector.tensor_tensor(acc[:, lo_:ST],
                                                        acc[:, lo_:ST],
                                                        wt_[:, lo_:ST], ADD)
                        del acc_defer[:]

                    for idx, kt in enumerate(order):
                        m = kt - j * TJ
                        lo = m * 128 if m > 0 else 0
                        ps = pss.tile([128, ST], F32, tag="ps")
                        nc.tensor.matmul(
                            ps[:, lo:ST], k_sb[:, kh, kt * 128:(kt + 1) * 128],
                            qjt[:, lo:ST], start=True, stop=True)
                        if len(pend_av) >= 3:
                            fire_av()
                        wtile = wtp.tile([128, ST], BF16, tag="wt")
                        if m >= 0:
                            nc.vector.tensor_tensor(
                                ps[:, lo:lo + 128], ps[:, lo:lo + 128],
                                tri_sb[:], ADD)
                        nc.scalar.activation(wtile[:, lo:ST], ps[:, lo:ST],
                                             EXP, scale=scale)
                        # Denominator: accumulate exp tiles on DVE (all-bf16
                        # hits the fast mode); one ones-matmul per head does
                        # the partition-dim sum, keeping 320 matmuls off PE.
                        # Diag-tile adds are DEFERRED: issued per-tile they
                        # would queue in front of the next diag tile's mask
                        # in the DVE FIFO (acc waits on exp) and serialize
                        # the whole mask->exp->acc chain.
                        if idx < TJ:
                            acc_defer.append((idx == 0, wtile, lo))
                        else:
                            flush_acc()
                            nc.vector.tensor_tensor(acc[:, lo:ST],
                                                    acc[:, lo:ST],
                                                    wtile[:, lo:ST], ADD)
                        pend_av.append((wtile, kt, lo))
                    flush_acc()

                    # The tail (remaining AVs + den + normalize chain) is
                    # returned as a closure so the caller can slot PE work
                    # (next block's q-projection) before it — the tail AVs
                    # then never wait on exp.
                    def tail():
                        while pend_av:
                            fire_av()
                        nc.tensor.matmul(pden[:], ones_col, acc[:],
                                         start=True, stop=True)
                        dns = rcpp.tile([1, ST], F32, tag="dns")
                        nc.vector.tensor_copy(dns[:], pden[:])
                        rcf = rcpp.tile([1, ST], F32, tag="rcf")
                        nc.vector.reciprocal_approx_fast(rcf[:], dns[:])
                        rcr = rcpp.tile([1, ST], BF16, tag="rcr")
                        nc.scalar.activation(rcr[:], rcf[:], COPY)
                        flush_epilogue()
                        pending.append((h, j, po, rcr))
                    return tail

                def load_wq(hg):
                    tiles = []
                    for hl in range(HG):
                        if hg == 0 and hl == 0:
                            tiles.append(wq_pre)
                            continue
                        pool_ = wqp0 if hl == 0 else wqp
                        wt_ = pool_.tile([128, DT, H], BF16,
                                         tag="wq%d" % hl,
                                         name="wq_%d_%d" % (hg, hl))
                        nc.sync.dma_start(wt_[:], wq.ap()[hg * HG + hl])
                        tiles.append(wt_)
                    return tiles

                NHG = QH // HG
                wqs_by_hg = {0: load_wq(0)}
                for bi in range(NB):
                    hg, j = sched[bi]
                    if bi % NJ == 0:
                        for hl in range(HG):
                            qproj_head(bi, hl, wqs_by_hg[hg])
                    if bi == NJ - 1 and NHG > 1:
                        wqs_by_hg[1] = load_wq(1)
                    nxt = bi + 1
                    interleave = nxt < NB and nxt % NJ != 0
                    if bi == NB - 1:
                        # Prefetch the first o-proj weight tile; its first
                        # four column groups (s-tiles finished blocks ago)
                        # interleave into this last block's attention in
                        # place of a next q-projection.
                        woe0 = wop.tile([128, QH, EW], BF16, tag="woe",
                                        name="woe_pre")
                        for h in range(QH):
                            nc.sync.dma_start(
                                woe0[:, h, :], wo.ap()[h, :, 0:EW])

                    def passC_group(st):
                        pc = psq.tile([128, EW], F32, tag="pq", name="pcp")
                        for h in range(QH):
                            nc.tensor.matmul(
                                pc[:],
                                o_sb[:, h, st * 128:(st + 1) * 128],
                                woe0[:, h, :],
                                start=(h == 0), stop=(h == QH - 1))
                        oc = onp.tile([128, EW], BF16, tag="oc0", bufs=2)
                        nc.vector.tensor_copy(oc[:], pc[:])
                        nc.scalar.dma_start(
                            o_out[st * 128:(st + 1) * 128, 0:EW], oc[:])

                    for hl in range(HG):
                        tail = attn_head(hg * HG + hl, qj_store[bi][hl], j)
                        if interleave:
                            nhg, _ = sched[nxt]
                            qproj_head(nxt, hl, wqs_by_hg[nhg])
                        elif bi == NB - 1:
                            passC_group(S // 128 - 1 - 3 * hl)
                            passC_group(S // 128 - 2 - 3 * hl)
                            passC_group(S // 128 - 3 - 3 * hl)
                        tail()
                    del qj_store[bi]
                flush_epilogue()

            # ---- PASS C: o projection (partial over this core's heads) ----
            # Attention outputs are already normalized in SBUF (o_sb).
            with nc.named_scope("passC"), \
                 tc.tile_pool(name="ocp", bufs=3) as ocp, \
                 tc.tile_pool(name="psc", bufs=4, space="PSUM") as psc:
                NST = S // 128
                for e in range(NE):
                    if e == 0:
                        woe = woe0
                    else:
                        woe = wop.tile([128, QH, EW], BF16, tag="woe")
                        for h in range(QH):
                            nc.sync.dma_start(
                                woe[:, h, :],
                                wo.ap()[h, :, e * EW:(e + 1) * EW])
                    # Descending s-tiles: o_sb columns complete in that
                    # order (schedule runs j descending per head-group);
                    # the last twelve groups of e=0 ran inside pass B.
                    for st in reversed(range(NST - 12 if e == 0 else NST)):
                        pc = psc.tile([128, EW], F32, tag="pc")
                        for h in range(QH):
                            nc.tensor.matmul(
                                pc[:],
                                o_sb[:, h, st * 128:(st + 1) * 128],
                                woe[:, h, :],
                                start=(h == 0), stop=(h == QH - 1))
                        oc = ocp.tile([128, EW], BF16, tag="oc")
                        nc.vector.tensor_copy(oc[:], pc[:])
                        nc.scalar.dma_start(
                            o_out[st * 128:(st + 1) * 128, e * EW:(e + 1) * EW],
                            oc[:])

    nc.compile()
    return nc


def _perm_matrix():
    P = np.zeros((128, 128), dtype=np.float32)
    P[np.arange(128), (np.arange(128) + 64) % 128] = 1.0
    return P


def make_tables(positions_b, S, H):
    """cos/sin tables in [128, S] layout with the sign fold for the swap
    trick (rows 0:63 -> +sin, 64:127 -> -sin), plus the triangular mask."""
    half = H // 2
    inv_freq = 1.0 / (ROPE_THETA ** (np.arange(half, dtype=np.float64) * 2.0 / H))
    ang = positions_b.astype(np.float64)[None, :] * inv_freq[:, None]  # [half, S]
    cos_h = np.cos(ang)
    sin_h = np.sin(ang)
    import ml_dtypes
    cos_t = np.concatenate([cos_h, cos_h], axis=0).astype(ml_dtypes.bfloat16)
    sin_t = np.concatenate([sin_h, -sin_h], axis=0).astype(ml_dtypes.bfloat16)
    idx = np.arange(128)
    tri = np.where(idx[:, None] <= idx[None, :], 0.0, NEG_BIG).astype(np.float32)
    return cos_t, sin_t, tri


def make_in_maps(x, positions, Wq, Wk, Wv, Wo, cfg):
    """Shard the full inputs into the 8 per-core input maps."""
    QH, KH = cfg["QH"], cfg["KH"]
    S, H = cfg["S"], cfg["H"]
    B = x.shape[0]
    groups = N_CORES // B
    tables = [make_tables(np.asarray(positions[b]), S, H) for b in range(B)]
    in_maps = []
    for c in range(N_CORES):
        b, g = divmod(c, groups)
        cos_t, sin_t, tri = tables[b]
        import ml_dtypes
        bf = ml_dtypes.bfloat16

        def pmaj(w):
            # [heads, D, H] -> [heads, 128, D/128, H] (partition-major so
            # each SBUF partition's data is one contiguous run)
            n, d, hh = w.shape
            return np.ascontiguousarray(
                w.reshape(n, d // 128, 128, hh).transpose(0, 2, 1, 3))

        xT_ds = np.asarray(x[b]).T.astype(bf)          # [D, S]
        d_, s_ = xT_ds.shape
        xc = np.ascontiguousarray(
            xT_ds.reshape(d_ // 128, 128, s_ // 512, 512)
            .transpose(1, 2, 0, 3))                    # [128, S/512, D/128, 512]
        in_maps.append({
            "xT": xc,
            "wq": pmaj(Wq[g * QH:(g + 1) * QH].astype(bf)),
            "wk": pmaj(Wk[g * KH:(g + 1) * KH].astype(bf)),
            "wv": pmaj(Wv[g * KH:(g + 1) * KH].astype(bf)),
            "wo": np.ascontiguousarray(Wo[g * QH:(g + 1) * QH].astype(bf)),
            "cos_t": cos_t,
            "sin_t": sin_t,
            "tri_t": tri,
            "ones_t": np.ones((128, 128), dtype=bf),
            "perm_t": _perm_matrix().astype(bf),
        })
    return in_maps


_NC_CACHE = {}


def _get_nc(cfg_key=None):
    cfg = FULL_CFG if cfg_key is None else cfg_key
    key = tuple(sorted(cfg.items()))
    if key not in _NC_CACHE:
        _NC_CACHE[key] = build_bass(cfg)
    return _NC_CACHE[key]


def run(x, positions, Wq, Wk, Wv, Wo, trace=False, trace_kwargs=None):
    cfg = FULL_CFG
    nc = _get_nc(cfg)
    in_maps = make_in_maps(np.asarray(x), np.asarray(positions),
                           np.asarray(Wq), np.asarray(Wk), np.asarray(Wv),
                           np.asarray(Wo), cfg)
    res = bass_utils.run_bass_kernel_spmd(
        nc, in_maps, list(range(N_CORES)), trace=trace,
        **(trace_kwargs or {}))
    B = np.asarray(x).shape[0]
    groups = N_CORES // B
    outs = []
    for b in range(B):
        acc = res.results[b * groups]["o_out"].astype(np.float64)
        for g in range(1, groups):
            acc += res.results[b * groups + g]["o_out"]
        outs.append(acc.astype(np.float32))
    return np.stack(outs, axis=0), res


def kernel(x, positions, Wq, Wk, Wv, Wo):
    out, _ = run(x, positions, Wq, Wk, Wv, Wo, trace=False)
    return out

